# revision 8
# baseline (speedup 1.0000x reference)
import sys

sys.path.insert(0, "/opt/trn_rl_repo")

import numpy as np

N = 100000
D = 32
E = 1600000
NCORES = 8
ROWS_PER_CORE = N // NCORES  # 12500
P = 128
WINDOWS = (ROWS_PER_CORE + P - 1) // P  # 98
PADROW = 128          # f16 elements per table row (256B stride)
GCOLS_MAX = 192       # slot columns per SBUF group
GATH_COLS = 96        # slot columns per dma_gather (12288 cells)
CTCAP = GATH_COLS * P  # compact-table rows reserved per gather (12288)


def _host_pack(edge_row, edge_col, edge_val, wpad16):
    """Build per-core window grids, group/gather structure, compact tables.

    Rows are count-sorted per core; window w needs T_w = max slots (incl. a
    bias pseudo-edge col=N val=1 per real row). T is shared across cores so
    the SPMD program is identical. Slot grid cells hold (col, val); cells are
    gathered in pieces of <=8192 via InstDMAGatherAnt with per-piece
    compacted tables (int16 local indices).
    """
    edge_row = np.asarray(edge_row).astype(np.int64)
    edge_col = np.asarray(edge_col).astype(np.int64)
    edge_val = np.asarray(edge_val).astype(np.float32)

    core_of = edge_row // ROWS_PER_CORE
    per_core = []
    for c in range(NCORES):
        m = core_of == c
        r = edge_row[m] - c * ROWS_PER_CORE
        col = edge_col[m]
        val = edge_val[m]
        cnt = np.bincount(r, minlength=ROWS_PER_CORE)  # real edges per row
        order = np.argsort(-cnt, kind="stable")
        pos_of_row = np.empty(ROWS_PER_CORE, dtype=np.int64)
        pos_of_row[order] = np.arange(ROWS_PER_CORE)
        eorder = np.argsort(pos_of_row[r], kind="stable")
        r_s = pos_of_row[r[eorder]]
        col_s = col[eorder]
        val_s = val[eorder]
        starts = np.zeros(ROWS_PER_CORE + 1, dtype=np.int64)
        starts[1:] = np.cumsum(cnt[order])
        slot_idx = np.arange(len(r_s)) - starts[r_s]
        per_core.append(dict(order=order, cnt_sorted=cnt[order], r_s=r_s,
                             col_s=col_s, val_s=val_s, slot_idx=slot_idx))

    # global T_w (slots incl. bias) per window
    T = np.zeros(WINDOWS, dtype=np.int64)
    for c in range(NCORES):
        cs = per_core[c]["cnt_sorted"] + 1  # +1 bias slot per real row
        pad = np.zeros(WINDOWS * P, dtype=np.int64)
        pad[:ROWS_PER_CORE] = cs
        T = np.maximum(T, pad.reshape(WINDOWS, P).max(axis=1))
    T = np.maximum(T, 1)

    # group windows: whole windows, <= cap slot cols per group. Later
    # groups get progressively smaller caps so the pipeline tail (fill+
    # mult+reduce+out of the final groups) is short.
    rem = [int(T[w:].sum()) for w in range(WINDOWS)] + [0]

    def cap_for(w):
        r = rem[w]
        if r <= 12:
            return 12
        if r <= 36:
            return 24
        if r <= 84:
            return 48
        return GCOLS_MAX

    groups = []  # (w0, w1, col_off, gcols)
    w0, acc, col_off = 0, 0, 0
    for w in range(WINDOWS):
        if acc > 0 and acc + T[w] > cap_for(w0):
            groups.append((w0, w, col_off, acc))
            col_off += acc
            w0, acc = w, 0
        acc += int(T[w])
    groups.append((w0, WINDOWS, col_off, acc))
    S = int(T.sum())

    offs = np.zeros(WINDOWS + 1, dtype=np.int64)
    offs[1:] = np.cumsum(T)

    # per-gather piece list: (group_idx, piece_idx_in_group, col_a, ncols)
    pieces = []
    for gi, (wa, wb, ca, gc) in enumerate(groups):
        k = 0
        a = ca
        while a < ca + gc:
            n = min(GATH_COLS, ca + gc - a)
            if gi == 0:
                ramp = [16, 32, 64]
                if k < len(ramp):
                    n = min(ramp[k], n)
            pieces.append((gi, k, a, n))
            a += n
            k += 1
    npieces_per_group = [sum(1 for p in pieces if p[0] == gi)
                         for gi in range(len(groups))]

    # build grids + compact tables per core
    metas = []
    for c in range(NCORES):
        pc = per_core[c]
        cols_g = np.full((P, S), N, dtype=np.int64)   # default: bias row
        vals_g = np.zeros((P, S), dtype=np.float32)
        # real edges
        w_of = pc["r_s"] // P
        p_of = pc["r_s"] % P
        cidx = offs[w_of] + pc["slot_idx"]
        cols_g[p_of, cidx] = pc["col_s"]
        vals_g[p_of, cidx] = pc["val_s"]
        # bias slot per real row (position j < ROWS_PER_CORE)
        jrows = np.arange(ROWS_PER_CORE)
        bw = jrows // P
        bp = jrows % P
        bslot = offs[bw] + pc["cnt_sorted"]  # slot index cnt (0-based)
        cols_g[bp, bslot] = N
        vals_g[bp, bslot] = 1.0

        # per-piece compaction; one DRAM table param per piece so the
        # gather base offset is always 0 (int16 base+idx limit)
        idxw_parts = []
        in_map = {"vals": np.ascontiguousarray(vals_g.astype(np.float16))}
        for pi, (gi, k, a, ncols) in enumerate(pieces):
            cells = cols_g[:, a:a + ncols]       # [P, ncols]
            flat = cells.T.reshape(-1)           # i = c*128 + p order
            uniq, inv = np.unique(flat, return_inverse=True)
            assert len(uniq) <= CTCAP
            tab = np.zeros((CTCAP, PADROW), dtype=np.float16)
            tab[:len(uniq)] = wpad16[uniq]
            in_map[f"ctab{pi}"] = tab
            n = ncols * P
            n16 = n // 16
            wrapped = inv.astype(np.int16).reshape(n16, 16).T  # [16, n16]
            idxw_parts.append(np.tile(wrapped, (8, 1)))
        idxw = np.ascontiguousarray(np.concatenate(idxw_parts, axis=1))
        in_map["idxw"] = idxw
        metas.append(in_map)

    return per_core, metas, T, groups, pieces, npieces_per_group, offs, S


def _build_program(T, groups, pieces, offs, S):
    from concourse import bass, bacc, mybir
    import concourse.tile as tile
    import concourse.ap_utils as ap_utils

    def dma_gather_raw(gp, out_ap, in_ap, idxs_ap, num_idxs, elem_size,
                       elem_step):
        assert idxs_ap.dtype == mybir.dt.int16
        assert in_ap.dtype == out_ap.dtype
        assert ap_utils.ap_is_contiguous(in_ap.ap[1:])
        assert ap_utils.ap_is_contiguous(out_ap.ap[1:])
        assert ap_utils.ap_is_contiguous(idxs_ap.ap[1:])
        assert in_ap.ap[-1][1] == out_ap.ap[-1][1] == elem_size
        assert in_ap.ap[0][0] == elem_step
        stride_bytes = elem_step * mybir.dt.size(in_ap.dtype)
        assert stride_bytes % 256 == 0 and stride_bytes < 65280
        return gp.add_instruction(
            mybir.InstDMAGatherAnt(
                name=gp.bass.get_next_instruction_name(),
                ins=[*gp.lower_ap_dma(in_ap, for_custom_bir_dma=True),
                     gp.lower_ap(idxs_ap),
                     gp.lower_val_access(gp.to_reg(num_idxs))],
                outs=[gp.lower_ap(out_ap)],
                transpose=False,
                num_idxs=num_idxs,
                elem_size=elem_size,
                stride_bytes_256=stride_bytes // 256,
                gen_mode=0,
                single_packet=False,
                queue_num=0,
                sbuf_tokens_per_rank=0,
                sbuf_free_dim_per_rank=0,
                sbuf_free_dim_pad_per_rank=0,
                sbuf_byte_offset=0,
            ))

    nc = bacc.Bacc()
    ngroups = len(groups)
    ctab_p = [nc.declare_dram_parameter(f"ctab{pi}", [CTCAP, PADROW],
                                        mybir.dt.float16, isOutput=False)
              for pi in range(len(pieces))]
    idxw_p = nc.declare_dram_parameter("idxw", [P, 8 * S], mybir.dt.int16,
                                       isOutput=False)
    vals_p = nc.declare_dram_parameter("vals", [P, S], mybir.dt.float16,
                                       isOutput=False)
    out_p = nc.declare_dram_parameter("out", [WINDOWS * P, D],
                                      mybir.dt.float16, isOutput=True)

    piece_by_group = {}
    for pi, (gi, k, a, ncols) in enumerate(pieces):
        piece_by_group.setdefault(gi, []).append((pi, a, ncols))

    with tile.TileContext(nc) as tc:
        with tc.tile_pool(name="sbuf", bufs=3) as sbuf, \
             tc.tile_pool(name="msb", bufs=1) as msb:
            for gi, (wa, wb, ca, gc) in enumerate(groups):
                idx_sb = sbuf.tile([P, 8 * gc], mybir.dt.int16, tag="I", bufs=4)
                nc.sync.dma_start(out=idx_sb[:],
                                  in_=idxw_p[:, 8 * ca:8 * (ca + gc)])
                val_sb = sbuf.tile([P, gc], mybir.dt.float16, tag="W", bufs=4)
                nc.sync.dma_start(out=val_sb[:],
                                  in_=vals_p[:, ca:ca + gc])
                Tt = sbuf.tile([P, gc * D], mybir.dt.float16, tag="T", bufs=4)
                Vt = sbuf.tile([P, gc * D], mybir.dt.float16, tag="V")
                vt3 = Vt[:, 0:gc * D].rearrange("p (s d) -> p s d", d=D)
                for (pi, a, ncols) in piece_by_group[gi]:
                    rel = a - ca
                    out_ap = Tt[:, rel * D:(rel + ncols) * D].rearrange(
                        "p (c e) -> p c e", e=D)
                    dma_gather_raw(
                        nc.gpsimd, out_ap,
                        ctab_p[pi][:, 0:D],
                        idx_sb[:, 8 * (a - ca):8 * (a - ca + ncols)],
                        ncols * P, D, PADROW)
                    # per-piece fill + multiply so DVE/Act start early
                    nc.scalar.copy(
                        out=vt3[:, rel:rel + ncols, 0:1].squeeze(2),
                        in_=val_sb[:, rel:rel + ncols])
                    k2 = 1
                    while k2 < D:
                        kk = min(k2, D - k2)
                        nc.scalar.copy(
                            out=vt3[:, rel:rel + ncols, k2:k2 + kk],
                            in_=vt3[:, rel:rel + ncols, 0:kk])
                        k2 += kk
                    nc.vector.tensor_tensor(
                        out=Tt[:, rel * D:(rel + ncols) * D],
                        in0=Tt[:, rel * D:(rel + ncols) * D],
                        in1=Vt[:, rel * D:(rel + ncols) * D],
                        op=mybir.AluOpType.mult)
                # reduce per window -> f32
                nw = wb - wa
                ot = sbuf.tile([P, nw * D], mybir.dt.float16, tag="O")
                with nc.allow_low_precision("f16 out; final rounding only"):
                    for w in range(wa, wb):
                        rel = int(offs[w]) - ca
                        Tw = int(T[w])
                        rin = Tt[:, rel * D:(rel + Tw) * D].rearrange(
                            "p (g d) -> p d g", d=D)
                        nc.vector.tensor_reduce(
                            out=ot[:, (w - wa) * D:(w - wa + 1) * D], in_=rin,
                            axis=mybir.AxisListType.X, op=mybir.AluOpType.add)
                oap = out_p[wa * P:wb * P, :].rearrange("(w p) d -> p w d",
                                                        p=P)
                nc.sync.dma_start(
                    out=oap, in_=ot[:].rearrange("p (w d) -> p w d", d=D))
    nc.compile()
    global _LAST_NC
    _LAST_NC = nc
    return nc


_LAST_NC = None


def kernel(edge_row, edge_col, edge_val, weight, bias):
    from concourse.bass_utils import run_bass_kernel_spmd

    weight = np.asarray(weight).astype(np.float32)
    bias = np.asarray(bias).astype(np.float32)
    wpad16 = np.zeros((N + 1, PADROW), dtype=np.float16)
    wpad16[:N, :D] = weight.astype(np.float16)
    wpad16[N, :D] = bias.astype(np.float16)

    (per_core, metas, T, groups, pieces, nppg, offs, S) = _host_pack(
        edge_row, edge_col, edge_val, wpad16)
    nc = _build_program(T, groups, pieces, offs, S)

    res = run_bass_kernel_spmd(nc, metas, list(range(NCORES)))

    out_full = np.empty((N, D), dtype=np.float32)
    for c in range(NCORES):
        oc = res.results[c]["out"]  # [WINDOWS*P, D] in sorted-row order
        order = per_core[c]["order"]
        out_full[c * ROWS_PER_CORE + order, :] = \
            oc[:ROWS_PER_CORE, :].astype(np.float32)
    return out_full


# revision 9
# speedup vs baseline: 1.0380x; 1.0380x over previous
import sys

sys.path.insert(0, "/opt/trn_rl_repo")

import numpy as np

N = 100000
D = 32
E = 1600000
NCORES = 8
ROWS_PER_CORE = N // NCORES  # 12500
P = 128
WINDOWS = (ROWS_PER_CORE + P - 1) // P  # 98
PADROW = 128          # f16 elements per table row (256B stride)
GCOLS_MAX = 192       # slot columns per SBUF group
GATH_COLS = 96        # slot columns per dma_gather (12288 cells)
CTCAP = GATH_COLS * P  # compact-table rows reserved per gather (12288)


def _host_pack(edge_row, edge_col, edge_val, wpad16):
    """Build per-core window grids, group/gather structure, compact tables.

    Rows are count-sorted per core; window w needs T_w = max slots (incl. a
    bias pseudo-edge col=N val=1 per real row). T is shared across cores so
    the SPMD program is identical. Slot grid cells hold (col, val); cells are
    gathered in pieces of <=8192 via InstDMAGatherAnt with per-piece
    compacted tables (int16 local indices).
    """
    edge_row = np.asarray(edge_row).astype(np.int64)
    edge_col = np.asarray(edge_col).astype(np.int64)
    edge_val = np.asarray(edge_val).astype(np.float32)

    core_of = edge_row // ROWS_PER_CORE
    per_core = []
    for c in range(NCORES):
        m = core_of == c
        r = edge_row[m] - c * ROWS_PER_CORE
        col = edge_col[m]
        val = edge_val[m]
        cnt = np.bincount(r, minlength=ROWS_PER_CORE)  # real edges per row
        order = np.argsort(-cnt, kind="stable")
        pos_of_row = np.empty(ROWS_PER_CORE, dtype=np.int64)
        pos_of_row[order] = np.arange(ROWS_PER_CORE)
        eorder = np.argsort(pos_of_row[r], kind="stable")
        r_s = pos_of_row[r[eorder]]
        col_s = col[eorder]
        val_s = val[eorder]
        starts = np.zeros(ROWS_PER_CORE + 1, dtype=np.int64)
        starts[1:] = np.cumsum(cnt[order])
        slot_idx = np.arange(len(r_s)) - starts[r_s]
        per_core.append(dict(order=order, cnt_sorted=cnt[order], r_s=r_s,
                             col_s=col_s, val_s=val_s, slot_idx=slot_idx))

    # global T_w (slots incl. bias) per window
    T = np.zeros(WINDOWS, dtype=np.int64)
    for c in range(NCORES):
        cs = per_core[c]["cnt_sorted"] + 1  # +1 bias slot per real row
        pad = np.zeros(WINDOWS * P, dtype=np.int64)
        pad[:ROWS_PER_CORE] = cs
        T = np.maximum(T, pad.reshape(WINDOWS, P).max(axis=1))
    T = np.maximum(T, 1)

    # group windows: whole windows, <= cap slot cols per group. Later
    # groups get progressively smaller caps so the pipeline tail (fill+
    # mult+reduce+out of the final groups) is short.
    rem = [int(T[w:].sum()) for w in range(WINDOWS)] + [0]

    def cap_for(w):
        r = rem[w]
        if r <= 12:
            return 12
        if r <= 36:
            return 24
        if r <= 84:
            return 48
        return GCOLS_MAX

    groups = []  # (w0, w1, col_off, gcols)
    w0, acc, col_off = 0, 0, 0
    for w in range(WINDOWS):
        if acc > 0 and acc + T[w] > cap_for(w0):
            groups.append((w0, w, col_off, acc))
            col_off += acc
            w0, acc = w, 0
        acc += int(T[w])
    groups.append((w0, WINDOWS, col_off, acc))
    S = int(T.sum())

    offs = np.zeros(WINDOWS + 1, dtype=np.int64)
    offs[1:] = np.cumsum(T)

    # per-gather piece list: (group_idx, piece_idx_in_group, col_a, ncols)
    pieces = []
    for gi, (wa, wb, ca, gc) in enumerate(groups):
        k = 0
        a = ca
        while a < ca + gc:
            n = min(GATH_COLS, ca + gc - a)
            if gi == 0:
                ramp = [16, 32, 64]
                if k < len(ramp):
                    n = min(ramp[k], n)
            pieces.append((gi, k, a, n))
            a += n
            k += 1
    npieces_per_group = [sum(1 for p in pieces if p[0] == gi)
                         for gi in range(len(groups))]

    # build grids + compact tables per core
    metas = []
    for c in range(NCORES):
        pc = per_core[c]
        cols_g = np.full((P, S), N, dtype=np.int64)   # default: bias row
        vals_g = np.zeros((P, S), dtype=np.float32)
        # real edges
        w_of = pc["r_s"] // P
        p_of = pc["r_s"] % P
        cidx = offs[w_of] + pc["slot_idx"]
        cols_g[p_of, cidx] = pc["col_s"]
        vals_g[p_of, cidx] = pc["val_s"]
        # bias slot per real row (position j < ROWS_PER_CORE)
        jrows = np.arange(ROWS_PER_CORE)
        bw = jrows // P
        bp = jrows % P
        bslot = offs[bw] + pc["cnt_sorted"]  # slot index cnt (0-based)
        cols_g[bp, bslot] = N
        vals_g[bp, bslot] = 1.0

        # per-piece compaction; one DRAM table param per piece so the
        # gather base offset is always 0 (int16 base+idx limit)
        idxw_parts = []
        in_map = {"vals": np.ascontiguousarray(vals_g.astype(np.float16))}
        for pi, (gi, k, a, ncols) in enumerate(pieces):
            cells = cols_g[:, a:a + ncols]       # [P, ncols]
            flat = cells.T.reshape(-1)           # i = c*128 + p order
            uniq, inv = np.unique(flat, return_inverse=True)
            assert len(uniq) <= CTCAP
            tab = np.zeros((CTCAP, PADROW), dtype=np.float16)
            tab[:len(uniq)] = wpad16[uniq]
            in_map[f"ctab{pi}"] = tab
            n = ncols * P
            n16 = n // 16
            wrapped = inv.astype(np.int16).reshape(n16, 16).T  # [16, n16]
            idxw_parts.append(np.tile(wrapped, (8, 1)))
        idxw = np.ascontiguousarray(np.concatenate(idxw_parts, axis=1))
        in_map["idxw"] = idxw
        metas.append(in_map)

    return per_core, metas, T, groups, pieces, npieces_per_group, offs, S


def _build_program(T, groups, pieces, offs, S):
    from concourse import bass, bacc, mybir
    import concourse.tile as tile
    import concourse.ap_utils as ap_utils

    def dma_gather_raw(gp, out_ap, in_ap, idxs_ap, num_idxs, elem_size,
                       elem_step):
        assert idxs_ap.dtype == mybir.dt.int16
        assert in_ap.dtype == out_ap.dtype
        assert ap_utils.ap_is_contiguous(in_ap.ap[1:])
        assert ap_utils.ap_is_contiguous(out_ap.ap[1:])
        assert ap_utils.ap_is_contiguous(idxs_ap.ap[1:])
        assert in_ap.ap[-1][1] == out_ap.ap[-1][1] == elem_size
        assert in_ap.ap[0][0] == elem_step
        stride_bytes = elem_step * mybir.dt.size(in_ap.dtype)
        assert stride_bytes % 256 == 0 and stride_bytes < 65280
        return gp.add_instruction(
            mybir.InstDMAGatherAnt(
                name=gp.bass.get_next_instruction_name(),
                ins=[*gp.lower_ap_dma(in_ap, for_custom_bir_dma=True),
                     gp.lower_ap(idxs_ap),
                     gp.lower_val_access(gp.to_reg(num_idxs))],
                outs=[gp.lower_ap(out_ap)],
                transpose=False,
                num_idxs=num_idxs,
                elem_size=elem_size,
                stride_bytes_256=stride_bytes // 256,
                gen_mode=0,
                single_packet=False,
                queue_num=0,
                sbuf_tokens_per_rank=0,
                sbuf_free_dim_per_rank=0,
                sbuf_free_dim_pad_per_rank=0,
                sbuf_byte_offset=0,
            ))

    nc = bacc.Bacc()
    ngroups = len(groups)
    ctab_p = [nc.declare_dram_parameter(f"ctab{pi}", [CTCAP, PADROW],
                                        mybir.dt.float16, isOutput=False)
              for pi in range(len(pieces))]
    idxw_p = nc.declare_dram_parameter("idxw", [P, 8 * S], mybir.dt.int16,
                                       isOutput=False)
    vals_p = nc.declare_dram_parameter("vals", [P, S], mybir.dt.float16,
                                       isOutput=False)
    out_p = nc.declare_dram_parameter("out", [WINDOWS * P, D],
                                      mybir.dt.float16, isOutput=True)

    piece_by_group = {}
    for pi, (gi, k, a, ncols) in enumerate(pieces):
        piece_by_group.setdefault(gi, []).append((pi, a, ncols))

    with tile.TileContext(nc) as tc:
        with tc.tile_pool(name="sbuf", bufs=3) as sbuf, \
             tc.tile_pool(name="msb", bufs=1) as msb:
            for gi, (wa, wb, ca, gc) in enumerate(groups):
                idx_sb = sbuf.tile([P, 8 * gc], mybir.dt.int16, tag="I", bufs=4)
                nc.sync.dma_start(out=idx_sb[:],
                                  in_=idxw_p[:, 8 * ca:8 * (ca + gc)])
                val_sb = sbuf.tile([P, gc], mybir.dt.float16, tag="W", bufs=4)
                nc.sync.dma_start(out=val_sb[:],
                                  in_=vals_p[:, ca:ca + gc])
                Tt = sbuf.tile([P, gc * D], mybir.dt.float16, tag="T", bufs=4)
                Vt = sbuf.tile([P, gc * D], mybir.dt.float16, tag="V")
                vt3 = Vt[:, 0:gc * D].rearrange("p (s d) -> p s d", d=D)
                for (pi, a, ncols) in piece_by_group[gi]:
                    rel = a - ca
                    out_ap = Tt[:, rel * D:(rel + ncols) * D].rearrange(
                        "p (c e) -> p c e", e=D)
                    dma_gather_raw(
                        nc.gpsimd, out_ap,
                        ctab_p[pi][:, 0:D],
                        idx_sb[:, 8 * (a - ca):8 * (a - ca + ncols)],
                        ncols * P, D, PADROW)
                # Act doubling fill of Vt, then one multiply per group
                nc.scalar.copy(out=vt3[:, :, 0:1].squeeze(2), in_=val_sb[:])
                k2 = 1
                while k2 < D:
                    kk = min(k2, D - k2)
                    nc.scalar.copy(out=vt3[:, :, k2:k2 + kk],
                                   in_=vt3[:, :, 0:kk])
                    k2 += kk
                nc.vector.tensor_tensor(out=Tt[:, 0:gc * D],
                                        in0=Tt[:, 0:gc * D],
                                        in1=Vt[:, 0:gc * D],
                                        op=mybir.AluOpType.mult)
                # reduce per window -> f32
                nw = wb - wa
                ot = sbuf.tile([P, nw * D], mybir.dt.float16, tag="O")
                with nc.allow_low_precision("f16 out; final rounding only"):
                    for w in range(wa, wb):
                        rel = int(offs[w]) - ca
                        Tw = int(T[w])
                        rin = Tt[:, rel * D:(rel + Tw) * D].rearrange(
                            "p (g d) -> p d g", d=D)
                        nc.vector.tensor_reduce(
                            out=ot[:, (w - wa) * D:(w - wa + 1) * D], in_=rin,
                            axis=mybir.AxisListType.X, op=mybir.AluOpType.add)
                oap = out_p[wa * P:wb * P, :].rearrange("(w p) d -> p w d",
                                                        p=P)
                nc.sync.dma_start(
                    out=oap, in_=ot[:].rearrange("p (w d) -> p w d", d=D))
    nc.compile()
    global _LAST_NC
    _LAST_NC = nc
    return nc


_LAST_NC = None


def kernel(edge_row, edge_col, edge_val, weight, bias):
    from concourse.bass_utils import run_bass_kernel_spmd

    weight = np.asarray(weight).astype(np.float32)
    bias = np.asarray(bias).astype(np.float32)
    wpad16 = np.zeros((N + 1, PADROW), dtype=np.float16)
    wpad16[:N, :D] = weight.astype(np.float16)
    wpad16[N, :D] = bias.astype(np.float16)

    (per_core, metas, T, groups, pieces, nppg, offs, S) = _host_pack(
        edge_row, edge_col, edge_val, wpad16)
    nc = _build_program(T, groups, pieces, offs, S)

    res = run_bass_kernel_spmd(nc, metas, list(range(NCORES)))

    out_full = np.empty((N, D), dtype=np.float32)
    for c in range(NCORES):
        oc = res.results[c]["out"]  # [WINDOWS*P, D] in sorted-row order
        order = per_core[c]["order"]
        out_full[c * ROWS_PER_CORE + order, :] = \
            oc[:ROWS_PER_CORE, :].astype(np.float32)
    return out_full
